# revision 12
# baseline (speedup 1.0000x reference)
"""Trainium2 Bass kernel for CausalAttention (sliding-window + scale-frame sparse attention).

Problem shape (hardcoded): B=1, N=4096, C=512, H=8, Dh=64, frame_seqlen=256,
sliding_window_size=2, num_frame_per_block=1, num_frame_for_scale=2.

Sharding: sequence-parallel over 8 NeuronCores. Core i owns queries
[512*i, 512*(i+1)) (= frames 2i, 2i+1) and returns that slice of the final
output. Keys needed per core: the 512 "scale" tokens (frames 0,1; attended by
every query unconditionally per the reference mask) plus a 3-frame window
{2i-1, 2i, 2i+1} (768 tokens). No collectives; host concatenates the slices.

Per-core device pipeline (all matmuls bf16 with fp32 PSUM accumulation):
  1. QKV projection in transposed layout: QT/KT = W @ x^T (channels on
     partitions), V in natural [token, dh] layout with a ones-column per head
     appended (so the attention-value matmul also produces softmax sums).
  2. Scores computed transposed, S^T[k, q] = K @ Q^T, per head per key-tile.
  3. exp on ScalarE straight out of PSUM (softmax scale folded into the
     activation's `scale`; no max-subtraction needed: scores are O(10) so
     fp32 exp cannot overflow; this matches jax softmax to rounding error).
  4. Mask structure applied multiplicatively to the bf16 probabilities:
     whole-block validity flags (per-core data) and a tril mask for the
     diagonal frame.
  5. O^T = V'^T @ P^T accumulated over key tiles; row 64 of the accumulator
     holds the softmax denominators; normalize via DVE reciprocal + GpSimd
     partition-broadcast.
  6. out^T = Wproj^T.T @ O^T (+bias), DMA out. Host transposes + concats.
"""

from contextlib import ExitStack

import numpy as np
import ml_dtypes

N, C, H, DH = 4096, 512, 8, 64
F = 256                 # frame_seqlen
NCORES = 8
NQ = N // NCORES        # 512 queries per core (2 frames)
KS = 512                # scale tokens (frames 0,1)
KW = 3 * F              # window tokens per core
NK = KS + KW            # 1280 keys per core
BF16 = ml_dtypes.bfloat16

_CACHE = {}


def _build(repeat=1):
    """Build + compile the (single, SPMD) Bass program. Returns nc."""
    import concourse.bass as bass  # noqa: F401
    import concourse.mybir as mybir
    import concourse.tile as tile
    from concourse import bacc

    f32 = mybir.dt.float32
    bf16 = mybir.dt.bfloat16
    EXP = mybir.ActivationFunctionType.Exp

    nc = bacc.Bacc("TRN2", target_bir_lowering=False, debug=False)

    xT = nc.dram_tensor("xT", [C, NK], bf16, kind="ExternalInput")
    wqT = nc.dram_tensor("wqT", [C, C], bf16, kind="ExternalInput")
    wkT = nc.dram_tensor("wkT", [C, C], bf16, kind="ExternalInput")
    wvT = nc.dram_tensor("wvT", [C, C], bf16, kind="ExternalInput")
    wpT = nc.dram_tensor("wpT", [64, H * C], bf16, kind="ExternalInput")
    btab = nc.dram_tensor("btab", [128, 12], f32, kind="ExternalInput")
    dmsk = nc.dram_tensor("dmsk", [8 * 128, F], bf16, kind="ExternalInput")
    outT = nc.dram_tensor("outT", [C, NQ], f32, kind="ExternalOutput")

    with tile.TileContext(nc) as tc, ExitStack() as ctx:
        cp = ctx.enter_context(tc.tile_pool(name="const", bufs=1))
        dp = ctx.enter_context(tc.tile_pool(name="data", bufs=1))
        ptp = ctx.enter_context(tc.tile_pool(name="pt", bufs=5))
        recp = ctx.enter_context(tc.tile_pool(name="rec", bufs=2))
        ppp = ctx.enter_context(tc.tile_pool(name="pp", bufs=2, space="PSUM"))
        stp = ctx.enter_context(tc.tile_pool(name="st", bufs=6, space="PSUM"))

        def body():
            xs = cp.tile([128, 4, NK], bf16, tag="xs")
            wq = cp.tile([128, 4, C], bf16, tag="wq")
            wk = cp.tile([128, 4, C], bf16, tag="wk")
            wv = cp.tile([128, 4, C], bf16, tag="wv")
            xr = xT.ap().rearrange("(a p) t -> p a t", p=128)
            wqr = wqT.ap().rearrange("(a p) o -> p a o", p=128)
            wkr = wkT.ap().rearrange("(a p) o -> p a o", p=128)
            wvr = wvT.ap().rearrange("(a p) o -> p a o", p=128)
            for ci in range(4):
                nc.sync.dma_start(xs[:, ci, :], xr[:, ci, :])
                nc.sync.dma_start(wv[:, ci, :], wvr[:, ci, :])
                nc.sync.dma_start(wq[:, ci, :], wqr[:, ci, :])
                nc.sync.dma_start(wk[:, ci, :], wkr[:, ci, :])
            wp = cp.tile([64, H, C], bf16, tag="wp")
            nc.sync.dma_start(wp[:], wpT.ap().rearrange("p (h o) -> p h o", h=H))
            bt = cp.tile([128, 12], f32, tag="bt")
            nc.sync.dma_start(bt[:], btab.ap())
            dm = cp.tile([128, 8, F], bf16, tag="dm")
            nc.sync.dma_start(dm[:], dmsk.ap().rearrange("(a p) q -> p a q", p=128))

            QT = dp.tile([128, 4, NQ], bf16, tag="QT")
            KT = dp.tile([128, 4, NK], bf16, tag="KT")
            V = dp.tile([128, 10, H, DH + 1], bf16, tag="V")
            OT = dp.tile([128, H, NQ], bf16, tag="OT")
            oT = dp.tile([128, 4, NQ], f32, tag="oT")

            # ---- V projection (natural layout) + ones column ----
            def qkv_psum(n):
                # st pool is idle during the projection phase; steal its slots
                return (ppp.tile([128, 512], f32, tag="pp", name="qps")
                        if n % 2 == 0
                        else stp.tile([128, 512], f32, tag="st", name="qps"))

            nc.vector.memset(V[:, :, :, DH:DH + 1], 1.0)
            for tt in range(10):
                ps = qkv_psum(tt)
                for ci in range(4):
                    nc.tensor.matmul(ps[:], lhsT=xs[:, ci, 128 * tt:128 * (tt + 1)],
                                     rhs=wv[:, ci, :], start=(ci == 0), stop=(ci == 3))
                nc.scalar.copy(V[:, tt, :, 0:DH],
                               ps[:].rearrange("p (h d) -> p h d", h=H))

            # ---- Q^T projection (queries = window tokens 256:768) ----
            for ot in range(4):
                ps = qkv_psum(ot)
                for ci in range(4):
                    nc.tensor.matmul(ps[:], lhsT=wq[:, ci, 128 * ot:128 * (ot + 1)],
                                     rhs=xs[:, ci, KS + F:KS + F + NQ],
                                     start=(ci == 0), stop=(ci == 3))
                nc.vector.tensor_scalar_add(QT[:, ot, :], ps[:], bt[:, ot:ot + 1])

            # ---- K^T projection (all 1280 tokens) ----
            for ot in range(4):
                for ti, (t0, t1) in enumerate(((0, 512), (512, 1024), (1024, 1280))):
                    ps = qkv_psum(ot + ti)
                    for ci in range(4):
                        nc.tensor.matmul(ps[:, 0:t1 - t0],
                                         lhsT=wk[:, ci, 128 * ot:128 * (ot + 1)],
                                         rhs=xs[:, ci, t0:t1],
                                         start=(ci == 0), stop=(ci == 3))
                    nc.scalar.activation(KT[:, ot, t0:t1], ps[:, 0:t1 - t0],
                                          mybir.ActivationFunctionType.Identity,
                                          bias=bt[:, 4 + ot:5 + ot])

            # ---- attention: software-pipelined over heads so the PE
            # streams ST(h+1) while ACT drains exp(h) ----
            def emit_scores(h):
                po = h // 2
                prow = slice((h % 2) * 64, (h % 2) * 64 + 64)
                qh = QT[prow, po, :]
                # scale keys: k-tiles 0..3, all queries; 1-bank ST tiles keep
                # the exp pipeline deep
                pts = ptp.tile([128, 4, NQ], bf16, tag="pts", name="pts")
                for kt in range(4):
                    st = stp.tile([128, 512], f32, tag="st", name="st")
                    nc.tensor.matmul(st[:],
                                     lhsT=KT[prow, po, 128 * kt:128 * (kt + 1)],
                                     rhs=qh, start=True, stop=True)
                    nc.scalar.activation(pts[:, kt, :], st[:], EXP,
                                         scale=float(DH) ** -0.5)
                # window keys: per query-frame r, k-tiles {4+2r .. 7+2r}
                ptws = []
                for r in range(2):
                    ptw = ptp.tile([128, 4, 256], bf16, tag="ptw", name="ptw")
                    for g in range(2):
                        stw = stp.tile([128, 2, 256], f32, tag="st", name="stw")
                        for j in range(2):
                            kt = 4 + 2 * r + 2 * g + j
                            nc.tensor.matmul(stw[:, j, :],
                                             lhsT=KT[prow, po, 128 * kt:128 * (kt + 1)],
                                             rhs=qh[:, 256 * r:256 * (r + 1)],
                                             start=True, stop=True)
                        nc.scalar.activation(ptw[:, 2 * g:2 * g + 2, :], stw[:], EXP,
                                             scale=float(DH) ** -0.5)
                    # one combined multiplicative mask: [vf, vf, tril*vd] blocks
                    nc.vector.tensor_mul(ptw[:], ptw[:], dm[:, 4 * r:4 * r + 4, :])
                    ptws.append(ptw)
                return pts, ptws

            def emit_av(h, pts, ptws):
                # O^T accumulation (+ sums in row 64 via the ones column);
                # scale k-tiles cover the full query range in one N=512 pass
                av = ppp.tile([128, 512], f32, tag="pp", name="av")
                for kt in range(4):
                    nc.tensor.matmul(av[0:65, :], lhsT=V[:, kt, h, :],
                                     rhs=pts[:, kt, :],
                                     start=(kt == 0), stop=False,
                                     skip_group_check=True)
                for r in range(2):
                    for j in range(4):
                        kt = 4 + 2 * r + j
                        nc.tensor.matmul(av[0:65, 256 * r:256 * (r + 1)],
                                         lhsT=V[:, kt, h, :],
                                         rhs=ptws[r][:, j, :],
                                         start=False, stop=(j == 3),
                                         skip_group_check=True)
                # free av fast: stage unnormalized O^T + pull sums out, then
                # normalize out of a [128, 4] reshape (a [1, 512] DVE
                # reciprocal runs on one lane and costs ~3.4us)
                nc.scalar.copy(OT[0:64, h, :], av[0:64, :])
                sm = recp.tile([65, NQ], f32, tag="sm", name="sm")
                nc.vector.tensor_copy(sm[64:65, :], av[64:65, :])
                rs = recp.tile([128, 4], f32, tag="rs", name="rs")
                nc.sync.dma_start(rs[:], sm[64:65, :])
                nc.vector.reciprocal(rs[:], rs[:])
                rcb = recp.tile([64, NQ], f32, tag="rcb", name="rcb")
                nc.sync.dma_start(rcb[0:1, :], rs[:])
                nc.gpsimd.partition_broadcast(rcb[:, :], rcb[0:1, :])
                nc.vector.tensor_mul(OT[0:64, h, :], OT[0:64, h, :],
                                     rcb[0:64, :])

            pend = {}
            for h in range(8):
                pend[h] = emit_scores(h)
                if h >= 1:
                    emit_av(h - 1, *pend.pop(h - 1))
            emit_av(7, *pend.pop(7))

            # ---- output projection ----
            od = outT.ap().rearrange("(a p) q -> p a q", p=128)
            for ot in range(4):
                pj = ppp.tile([128, 512], f32, tag="pp")
                for h in range(8):
                    nc.tensor.matmul(pj[:],
                                     lhsT=wp[0:64, h, 128 * ot:128 * (ot + 1)],
                                     rhs=OT[0:64, h, :], start=(h == 0), stop=(h == 7))
                nc.vector.tensor_scalar_add(oT[:, ot, :], pj[:], bt[:, 8 + ot:9 + ot])
                nc.sync.dma_start(od[:, ot, :], oT[:, ot, :])

        if repeat == 1:
            body()
        else:
            with tc.For_i(0, repeat, 1):
                body()

    nc.compile()
    return nc


def _get_nc(repeat=1):
    key = ("nc", repeat)
    if key not in _CACHE:
        _CACHE[key] = _build(repeat)
    return _CACHE[key]


def _host_prep(x, qkv_w, qkv_b, proj_w, proj_b):
    """Build the 8 per-core input maps."""
    x = np.asarray(x, np.float32).reshape(N, C)
    qkv_w = np.asarray(qkv_w, np.float32)
    qkv_b = np.asarray(qkv_b, np.float32)
    proj_w = np.asarray(proj_w, np.float32)
    proj_b = np.asarray(proj_b, np.float32)

    xs_bf = x.astype(BF16)
    xT_scale = np.ascontiguousarray(xs_bf[0:KS].T)            # [C, 512]
    wqT = np.ascontiguousarray(qkv_w[0:C].T.astype(BF16))
    wkT = np.ascontiguousarray(qkv_w[C:2 * C].T.astype(BF16))
    wvT = np.ascontiguousarray(qkv_w[2 * C:3 * C].T.astype(BF16))
    # head-chunked proj weight: wpT[p, h*512+o] = proj_w[o, h*64+p] so every
    # head's 64-row contraction chunk sits at partition base 0
    wpT = np.ascontiguousarray(
        proj_w.T.reshape(H, 64, C).transpose(1, 0, 2).reshape(64, H * C).astype(BF16))

    # value-bias folds through normalized attention into the proj bias:
    # O = sum_k phat_k (V_k + vb) = O_hat + vb, so out += vb @ proj_w.T
    pb_eff = proj_b + qkv_b[2 * C:3 * C] @ proj_w.T
    btab = np.zeros((128, 12), np.float32)
    for ot in range(4):
        btab[:, ot] = qkv_b[0:C][128 * ot:128 * (ot + 1)]
        btab[:, 4 + ot] = qkv_b[C:2 * C][128 * ot:128 * (ot + 1)]
        btab[:, 8 + ot] = pb_eff[128 * ot:128 * (ot + 1)]

    # tril01[j, q] = 1 if key j <= query q (within the same frame)
    tril01 = (np.arange(F)[:, None] <= np.arange(F)[None, :])

    in_maps = []
    for i in range(NCORES):
        win = np.zeros((KW, C), BF16)
        lo = F * (2 * i - 1)
        src = xs_bf[max(0, lo):F * (2 * i + 2)]
        win[KW - len(src):] = src
        xTi = np.empty((C, NK), BF16)
        xTi[:, 0:KS] = xT_scale
        xTi[:, KS:] = win.T

        vf = np.array([1.0 if (2 * i - 1) >= 2 else 0.0,
                       1.0 if (2 * i) >= 2 else 0.0], np.float32)
        vd = np.array([1.0 if (2 * i) >= 2 else 0.0,
                       1.0 if (2 * i + 1) >= 2 else 0.0], np.float32)
        dmsk = np.zeros((8 * 128, F), np.float32)
        for r in range(2):
            for j in range(4):
                blk = dmsk[(4 * r + j) * 128:(4 * r + j + 1) * 128]
                if j < 2:
                    blk[:] = vf[r]
                else:
                    kh = j - 2
                    blk[:] = tril01[128 * kh:128 * (kh + 1), :] * vd[r]
        in_maps.append({
            "xT": xTi, "wqT": wqT, "wkT": wkT, "wvT": wvT, "wpT": wpT,
            "btab": btab,
            "dmsk": dmsk.astype(BF16),
        })
    return in_maps


def _check_fixed_params(block_mask, video_mask, frame_seqlen,
                        sliding_window_size, num_frame_per_block,
                        num_frame_for_scale):
    if int(frame_seqlen) != F or int(sliding_window_size) != 2 \
            or int(num_frame_per_block) != 1 or int(num_frame_for_scale) != 2:
        return False
    vm = np.asarray(video_mask)
    if not bool(vm.all()):
        return False
    bm = np.asarray(block_mask)
    if bm.shape != (N, N):
        return False
    # spot-check causality structure of block_mask (full check is 16M bools)
    idx = np.linspace(0, N - 1, 64).astype(int)
    sub = bm[np.ix_(idx, idx)]
    if not np.array_equal(sub, np.tril(np.ones_like(sub))):
        return False
    return True


def _numpy_reference(x, block_mask, video_mask, qkv_w, qkv_b, proj_w, proj_b,
                     frame_seqlen, sliding_window_size, num_frame_per_block,
                     num_frame_for_scale):
    """Fallback: direct numpy evaluation of the reference semantics."""
    x = np.asarray(x, np.float32)
    b, n, c = x.shape
    dh = c // H
    qkv = (x @ np.asarray(qkv_w).T + np.asarray(qkv_b)).reshape(b, n, 3, H, dh)
    qkv = qkv.transpose(2, 0, 3, 1, 4)
    q, k, v = qkv[0], qkv[1], qkv[2]
    mask = np.asarray(block_mask)[:n, :n][None, None]
    vm = np.asarray(video_mask)[:, None, None, None]
    mask = mask | ~vm
    fs = int(frame_seqlen)
    if int(sliding_window_size) > 0 and fs is not None:
        f = np.arange(n) // fs
        w = int(sliding_window_size) * int(num_frame_per_block)
        sliding = (f[None, :] <= f[:, None]) & (f[None, :] >= f[:, None] - w + 1)
        mask = mask & sliding[None, None]
        if int(num_frame_for_scale) > 0:
            s = int(num_frame_for_scale) * fs
            mask = mask.copy()
            mask[:, :, :, :s] = True
    scores = np.einsum('bhqd,bhkd->bhqk', q, k) * (dh ** -0.5)
    scores = np.where(mask, scores, np.float32(-1e30))
    scores -= scores.max(axis=-1, keepdims=True)
    e = np.exp(scores)
    attn = e / e.sum(axis=-1, keepdims=True)
    o = np.einsum('bhqk,bhkd->bhqd', attn, v)
    o = o.transpose(0, 2, 1, 3).reshape(b, n, c)
    return (o @ np.asarray(proj_w).T + np.asarray(proj_b)).astype(np.float32)


def kernel(x, block_mask, video_mask, qkv_w, qkv_b, proj_w, proj_b,
           frame_seqlen, sliding_window_size, num_frame_per_block,
           num_frame_for_scale):
    if not _check_fixed_params(block_mask, video_mask, frame_seqlen,
                               sliding_window_size, num_frame_per_block,
                               num_frame_for_scale):
        return _numpy_reference(x, block_mask, video_mask, qkv_w, qkv_b,
                                proj_w, proj_b, frame_seqlen,
                                sliding_window_size, num_frame_per_block,
                                num_frame_for_scale)

    from concourse.bass_utils import run_bass_kernel_spmd

    nc = _get_nc()
    in_maps = _host_prep(x, qkv_w, qkv_b, proj_w, proj_b)
    res = run_bass_kernel_spmd(nc, in_maps, core_ids=list(range(NCORES)))
    out = np.empty((N, C), np.float32)
    for i in range(NCORES):
        out[NQ * i:NQ * (i + 1)] = res.results[i]["outT"].T
    return out.reshape(1, N, C)


# revision 13
# speedup vs baseline: 1.0645x; 1.0645x over previous
"""Trainium2 Bass kernel for CausalAttention (sliding-window + scale-frame sparse attention).

Problem shape (hardcoded): B=1, N=4096, C=512, H=8, Dh=64, frame_seqlen=256,
sliding_window_size=2, num_frame_per_block=1, num_frame_for_scale=2.

Sharding: sequence-parallel over 8 NeuronCores. Core i owns queries
[512*i, 512*(i+1)) (= frames 2i, 2i+1) and returns that slice of the final
output. Keys needed per core: the 512 "scale" tokens (frames 0,1; attended by
every query unconditionally per the reference mask) plus a 3-frame window
{2i-1, 2i, 2i+1} (768 tokens). No collectives; host concatenates the slices.

Per-core device pipeline (all matmuls bf16 with fp32 PSUM accumulation):
  1. QKV projection in transposed layout: QT/KT = W @ x^T (channels on
     partitions), V in natural [token, dh] layout with a ones-column per head
     appended (so the attention-value matmul also produces softmax sums).
  2. Scores computed transposed, S^T[k, q] = K @ Q^T, per head per key-tile.
  3. exp on ScalarE straight out of PSUM (softmax scale folded into the
     activation's `scale`; no max-subtraction needed: scores are O(10) so
     fp32 exp cannot overflow; this matches jax softmax to rounding error).
  4. Mask structure applied multiplicatively to the bf16 probabilities:
     whole-block validity flags (per-core data) and a tril mask for the
     diagonal frame.
  5. O^T = V'^T @ P^T accumulated over key tiles; row 64 of the accumulator
     holds the softmax denominators; normalize via DVE reciprocal + GpSimd
     partition-broadcast.
  6. out^T = Wproj^T.T @ O^T (+bias), DMA out. Host transposes + concats.
"""

from contextlib import ExitStack

import numpy as np
import ml_dtypes

N, C, H, DH = 4096, 512, 8, 64
F = 256                 # frame_seqlen
NCORES = 8
NQ = N // NCORES        # 512 queries per core (2 frames)
KS = 512                # scale tokens (frames 0,1)
KW = 3 * F              # window tokens per core
NK = KS + KW            # 1280 keys per core
BF16 = ml_dtypes.bfloat16

_CACHE = {}


def _build(repeat=1):
    """Build + compile the (single, SPMD) Bass program. Returns nc."""
    import concourse.bass as bass  # noqa: F401
    import concourse.mybir as mybir
    import concourse.tile as tile
    from concourse import bacc

    f32 = mybir.dt.float32
    bf16 = mybir.dt.bfloat16
    EXP = mybir.ActivationFunctionType.Exp

    nc = bacc.Bacc("TRN2", target_bir_lowering=False, debug=False)

    xT = nc.dram_tensor("xT", [C, NK], bf16, kind="ExternalInput")
    wqT = nc.dram_tensor("wqT", [C, C], bf16, kind="ExternalInput")
    wkT = nc.dram_tensor("wkT", [C, C], bf16, kind="ExternalInput")
    wvT = nc.dram_tensor("wvT", [C, C], bf16, kind="ExternalInput")
    wpT = nc.dram_tensor("wpT", [64, H * C], bf16, kind="ExternalInput")
    btab = nc.dram_tensor("btab", [128, 12], f32, kind="ExternalInput")
    dmsk = nc.dram_tensor("dmsk", [8 * 128, F], bf16, kind="ExternalInput")
    outT = nc.dram_tensor("outT", [C, NQ], f32, kind="ExternalOutput")

    with tile.TileContext(nc) as tc, ExitStack() as ctx:
        cp = ctx.enter_context(tc.tile_pool(name="const", bufs=1))
        dp = ctx.enter_context(tc.tile_pool(name="data", bufs=1))
        ptp = ctx.enter_context(tc.tile_pool(name="pt", bufs=8))
        recp = ctx.enter_context(tc.tile_pool(name="rec", bufs=2))
        ppp = ctx.enter_context(tc.tile_pool(name="pp", bufs=2, space="PSUM"))
        stp = ctx.enter_context(tc.tile_pool(name="st", bufs=6, space="PSUM"))

        def body():
            xs = cp.tile([128, 4, NK], bf16, tag="xs")
            wq = cp.tile([128, 4, C], bf16, tag="wq")
            wk = cp.tile([128, 4, C], bf16, tag="wk")
            wv = cp.tile([128, 4, C], bf16, tag="wv")
            xr = xT.ap().rearrange("(a p) t -> p a t", p=128)
            wqr = wqT.ap().rearrange("(a p) o -> p a o", p=128)
            wkr = wkT.ap().rearrange("(a p) o -> p a o", p=128)
            wvr = wvT.ap().rearrange("(a p) o -> p a o", p=128)
            for ci in range(4):
                nc.sync.dma_start(xs[:, ci, :], xr[:, ci, :])
                nc.sync.dma_start(wv[:, ci, :], wvr[:, ci, :])
                nc.sync.dma_start(wq[:, ci, :], wqr[:, ci, :])
                nc.sync.dma_start(wk[:, ci, :], wkr[:, ci, :])
            wp = cp.tile([64, H, C], bf16, tag="wp")
            nc.sync.dma_start(wp[:], wpT.ap().rearrange("p (h o) -> p h o", h=H))
            bt = cp.tile([128, 12], f32, tag="bt")
            nc.sync.dma_start(bt[:], btab.ap())
            dm = cp.tile([128, 8, F], bf16, tag="dm")
            nc.sync.dma_start(dm[:], dmsk.ap().rearrange("(a p) q -> p a q", p=128))

            QT = dp.tile([128, 4, NQ], bf16, tag="QT")
            KT = dp.tile([128, 4, NK], bf16, tag="KT")
            V = dp.tile([128, 10, H, DH + 1], bf16, tag="V")
            OT = dp.tile([128, H, NQ], bf16, tag="OT")
            oT = dp.tile([128, 4, NQ], f32, tag="oT")

            # ---- V projection (natural layout) + ones column ----
            def qkv_psum(n):
                # st pool is idle during the projection phase; steal its slots
                return (ppp.tile([128, 512], f32, tag="pp", name="qps")
                        if n % 2 == 0
                        else stp.tile([128, 512], f32, tag="st", name="qps"))

            nc.vector.memset(V[:, :, :, DH:DH + 1], 1.0)
            for tt in range(10):
                ps = qkv_psum(tt)
                for ci in range(4):
                    nc.tensor.matmul(ps[:], lhsT=xs[:, ci, 128 * tt:128 * (tt + 1)],
                                     rhs=wv[:, ci, :], start=(ci == 0), stop=(ci == 3))
                nc.scalar.copy(V[:, tt, :, 0:DH],
                               ps[:].rearrange("p (h d) -> p h d", h=H))

            # ---- Q^T projection (queries = window tokens 256:768) ----
            for ot in range(4):
                ps = qkv_psum(ot)
                for ci in range(4):
                    nc.tensor.matmul(ps[:], lhsT=wq[:, ci, 128 * ot:128 * (ot + 1)],
                                     rhs=xs[:, ci, KS + F:KS + F + NQ],
                                     start=(ci == 0), stop=(ci == 3))
                nc.vector.tensor_scalar_add(QT[:, ot, :], ps[:], bt[:, ot:ot + 1])

            # ---- K^T projection (all 1280 tokens) ----
            for ot in range(4):
                for ti, (t0, t1) in enumerate(((0, 512), (512, 1024), (1024, 1280))):
                    ps = qkv_psum(ot + ti)
                    for ci in range(4):
                        nc.tensor.matmul(ps[:, 0:t1 - t0],
                                         lhsT=wk[:, ci, 128 * ot:128 * (ot + 1)],
                                         rhs=xs[:, ci, t0:t1],
                                         start=(ci == 0), stop=(ci == 3))
                    nc.scalar.activation(KT[:, ot, t0:t1], ps[:, 0:t1 - t0],
                                          mybir.ActivationFunctionType.Identity,
                                          bias=bt[:, 4 + ot:5 + ot])

            # ---- attention ----
            # per-k-tile PT tiles keep cross-engine dependencies fine-grained:
            # each AV matmul waits only on its own k-tile's exp/mask.
            for h in range(8):
                po = h // 2
                prow = slice((h % 2) * 64, (h % 2) * 64 + 64)
                qh = QT[prow, po, :]
                # scale keys: k-tiles 0..3, all queries
                pts = []
                for kt in range(4):
                    st = stp.tile([128, 512], f32, tag="st", name="st")
                    nc.tensor.matmul(st[:],
                                     lhsT=KT[prow, po, 128 * kt:128 * (kt + 1)],
                                     rhs=qh, start=True, stop=True)
                    pt = ptp.tile([128, NQ], bf16, tag="pts", name="pt")
                    nc.scalar.activation(pt[:], st[:], EXP,
                                         scale=float(DH) ** -0.5)
                    pts.append(pt)
                # window keys: per query-frame r, k-tile pairs, masked in halves
                ptws = []
                for r in range(2):
                    for g in range(2):
                        stw = stp.tile([128, 2, 256], f32, tag="st", name="stw")
                        for j in range(2):
                            kt = 4 + 2 * r + 2 * g + j
                            nc.tensor.matmul(stw[:, j, :],
                                             lhsT=KT[prow, po, 128 * kt:128 * (kt + 1)],
                                             rhs=qh[:, 256 * r:256 * (r + 1)],
                                             start=True, stop=True)
                        ptw = ptp.tile([128, 2, 256], bf16, tag="ptw", name="ptw")
                        nc.scalar.activation(ptw[:], stw[:], EXP,
                                             scale=float(DH) ** -0.5)
                        nc.vector.tensor_mul(ptw[:], ptw[:],
                                             dm[:, 4 * r + 2 * g:4 * r + 2 * g + 2, :])
                        ptws.append(ptw)
                # O^T accumulation (+ sums in row 64 via the ones column)
                av = ppp.tile([128, 512], f32, tag="pp", name="av")
                for kt in range(4):
                    nc.tensor.matmul(av[0:65, :], lhsT=V[:, kt, h, :],
                                     rhs=pts[kt][:],
                                     start=(kt == 0), stop=False,
                                     skip_group_check=True)
                for r in range(2):
                    for g in range(2):
                        for j in range(2):
                            kt = 4 + 2 * r + 2 * g + j
                            nc.tensor.matmul(av[0:65, 256 * r:256 * (r + 1)],
                                             lhsT=V[:, kt, h, :],
                                             rhs=ptws[2 * r + g][:, j, :],
                                             start=False,
                                             stop=(g == 1 and j == 1),
                                             skip_group_check=True)
                # free av fast: stage unnormalized O^T + pull sums out, then
                # normalize via a [128, 4]-reshaped reciprocal (a [1, 512]
                # DVE reciprocal runs on one lane and costs ~3.4us)
                nc.scalar.copy(OT[0:64, h, :], av[0:64, :])
                sm = recp.tile([65, NQ], f32, tag="sm", name="sm")
                nc.vector.tensor_copy(sm[64:65, :], av[64:65, :])
                rs = recp.tile([128, 4], f32, tag="rs", name="rs")
                nc.sync.dma_start(rs[:], sm[64:65, :])
                nc.vector.reciprocal(rs[:], rs[:])
                rcb = recp.tile([64, NQ], f32, tag="rcb", name="rcb")
                nc.sync.dma_start(rcb[0:1, :], rs[:])
                nc.gpsimd.partition_broadcast(rcb[:, :], rcb[0:1, :])
                nc.vector.tensor_mul(OT[0:64, h, :], OT[0:64, h, :],
                                     rcb[0:64, :])

            # ---- output projection ----
            od = outT.ap().rearrange("(a p) q -> p a q", p=128)
            for ot in range(4):
                pj = ppp.tile([128, 512], f32, tag="pp")
                for h in range(8):
                    nc.tensor.matmul(pj[:],
                                     lhsT=wp[0:64, h, 128 * ot:128 * (ot + 1)],
                                     rhs=OT[0:64, h, :], start=(h == 0), stop=(h == 7))
                nc.vector.tensor_scalar_add(oT[:, ot, :], pj[:], bt[:, 8 + ot:9 + ot])
                nc.sync.dma_start(od[:, ot, :], oT[:, ot, :])

        if repeat == 1:
            body()
        else:
            with tc.For_i(0, repeat, 1):
                body()

    nc.compile()
    return nc


def _get_nc(repeat=1):
    key = ("nc", repeat)
    if key not in _CACHE:
        _CACHE[key] = _build(repeat)
    return _CACHE[key]


def _host_prep(x, qkv_w, qkv_b, proj_w, proj_b):
    """Build the 8 per-core input maps."""
    x = np.asarray(x, np.float32).reshape(N, C)
    qkv_w = np.asarray(qkv_w, np.float32)
    qkv_b = np.asarray(qkv_b, np.float32)
    proj_w = np.asarray(proj_w, np.float32)
    proj_b = np.asarray(proj_b, np.float32)

    xs_bf = x.astype(BF16)
    xT_scale = np.ascontiguousarray(xs_bf[0:KS].T)            # [C, 512]
    wqT = np.ascontiguousarray(qkv_w[0:C].T.astype(BF16))
    wkT = np.ascontiguousarray(qkv_w[C:2 * C].T.astype(BF16))
    wvT = np.ascontiguousarray(qkv_w[2 * C:3 * C].T.astype(BF16))
    # head-chunked proj weight: wpT[p, h*512+o] = proj_w[o, h*64+p] so every
    # head's 64-row contraction chunk sits at partition base 0
    wpT = np.ascontiguousarray(
        proj_w.T.reshape(H, 64, C).transpose(1, 0, 2).reshape(64, H * C).astype(BF16))

    # value-bias folds through normalized attention into the proj bias:
    # O = sum_k phat_k (V_k + vb) = O_hat + vb, so out += vb @ proj_w.T
    pb_eff = proj_b + qkv_b[2 * C:3 * C] @ proj_w.T
    btab = np.zeros((128, 12), np.float32)
    for ot in range(4):
        btab[:, ot] = qkv_b[0:C][128 * ot:128 * (ot + 1)]
        btab[:, 4 + ot] = qkv_b[C:2 * C][128 * ot:128 * (ot + 1)]
        btab[:, 8 + ot] = pb_eff[128 * ot:128 * (ot + 1)]

    # tril01[j, q] = 1 if key j <= query q (within the same frame)
    tril01 = (np.arange(F)[:, None] <= np.arange(F)[None, :])

    in_maps = []
    for i in range(NCORES):
        win = np.zeros((KW, C), BF16)
        lo = F * (2 * i - 1)
        src = xs_bf[max(0, lo):F * (2 * i + 2)]
        win[KW - len(src):] = src
        xTi = np.empty((C, NK), BF16)
        xTi[:, 0:KS] = xT_scale
        xTi[:, KS:] = win.T

        vf = np.array([1.0 if (2 * i - 1) >= 2 else 0.0,
                       1.0 if (2 * i) >= 2 else 0.0], np.float32)
        vd = np.array([1.0 if (2 * i) >= 2 else 0.0,
                       1.0 if (2 * i + 1) >= 2 else 0.0], np.float32)
        dmsk = np.zeros((8 * 128, F), np.float32)
        for r in range(2):
            for j in range(4):
                blk = dmsk[(4 * r + j) * 128:(4 * r + j + 1) * 128]
                if j < 2:
                    blk[:] = vf[r]
                else:
                    kh = j - 2
                    blk[:] = tril01[128 * kh:128 * (kh + 1), :] * vd[r]
        in_maps.append({
            "xT": xTi, "wqT": wqT, "wkT": wkT, "wvT": wvT, "wpT": wpT,
            "btab": btab,
            "dmsk": dmsk.astype(BF16),
        })
    return in_maps


def _check_fixed_params(block_mask, video_mask, frame_seqlen,
                        sliding_window_size, num_frame_per_block,
                        num_frame_for_scale):
    if int(frame_seqlen) != F or int(sliding_window_size) != 2 \
            or int(num_frame_per_block) != 1 or int(num_frame_for_scale) != 2:
        return False
    vm = np.asarray(video_mask)
    if not bool(vm.all()):
        return False
    bm = np.asarray(block_mask)
    if bm.shape != (N, N):
        return False
    # spot-check causality structure of block_mask (full check is 16M bools)
    idx = np.linspace(0, N - 1, 64).astype(int)
    sub = bm[np.ix_(idx, idx)]
    if not np.array_equal(sub, np.tril(np.ones_like(sub))):
        return False
    return True


def _numpy_reference(x, block_mask, video_mask, qkv_w, qkv_b, proj_w, proj_b,
                     frame_seqlen, sliding_window_size, num_frame_per_block,
                     num_frame_for_scale):
    """Fallback: direct numpy evaluation of the reference semantics."""
    x = np.asarray(x, np.float32)
    b, n, c = x.shape
    dh = c // H
    qkv = (x @ np.asarray(qkv_w).T + np.asarray(qkv_b)).reshape(b, n, 3, H, dh)
    qkv = qkv.transpose(2, 0, 3, 1, 4)
    q, k, v = qkv[0], qkv[1], qkv[2]
    mask = np.asarray(block_mask)[:n, :n][None, None]
    vm = np.asarray(video_mask)[:, None, None, None]
    mask = mask | ~vm
    fs = int(frame_seqlen)
    if int(sliding_window_size) > 0 and fs is not None:
        f = np.arange(n) // fs
        w = int(sliding_window_size) * int(num_frame_per_block)
        sliding = (f[None, :] <= f[:, None]) & (f[None, :] >= f[:, None] - w + 1)
        mask = mask & sliding[None, None]
        if int(num_frame_for_scale) > 0:
            s = int(num_frame_for_scale) * fs
            mask = mask.copy()
            mask[:, :, :, :s] = True
    scores = np.einsum('bhqd,bhkd->bhqk', q, k) * (dh ** -0.5)
    scores = np.where(mask, scores, np.float32(-1e30))
    scores -= scores.max(axis=-1, keepdims=True)
    e = np.exp(scores)
    attn = e / e.sum(axis=-1, keepdims=True)
    o = np.einsum('bhqk,bhkd->bhqd', attn, v)
    o = o.transpose(0, 2, 1, 3).reshape(b, n, c)
    return (o @ np.asarray(proj_w).T + np.asarray(proj_b)).astype(np.float32)


def kernel(x, block_mask, video_mask, qkv_w, qkv_b, proj_w, proj_b,
           frame_seqlen, sliding_window_size, num_frame_per_block,
           num_frame_for_scale):
    if not _check_fixed_params(block_mask, video_mask, frame_seqlen,
                               sliding_window_size, num_frame_per_block,
                               num_frame_for_scale):
        return _numpy_reference(x, block_mask, video_mask, qkv_w, qkv_b,
                                proj_w, proj_b, frame_seqlen,
                                sliding_window_size, num_frame_per_block,
                                num_frame_for_scale)

    from concourse.bass_utils import run_bass_kernel_spmd

    nc = _get_nc()
    in_maps = _host_prep(x, qkv_w, qkv_b, proj_w, proj_b)
    res = run_bass_kernel_spmd(nc, in_maps, core_ids=list(range(NCORES)))
    out = np.empty((N, C), np.float32)
    for i in range(NCORES):
        out[NQ * i:NQ * (i + 1)] = res.results[i]["outT"].T
    return out.reshape(1, N, C)


# revision 14
# speedup vs baseline: 1.1935x; 1.1212x over previous
"""Trainium2 Bass kernel for CausalAttention (sliding-window + scale-frame sparse attention).

Problem shape (hardcoded): B=1, N=4096, C=512, H=8, Dh=64, frame_seqlen=256,
sliding_window_size=2, num_frame_per_block=1, num_frame_for_scale=2.

Sharding: sequence-parallel over 8 NeuronCores. Core i owns queries
[512*i, 512*(i+1)) (= frames 2i, 2i+1) and returns that slice of the final
output. Keys needed per core: the 512 "scale" tokens (frames 0,1; attended by
every query unconditionally per the reference mask) plus a 3-frame window
{2i-1, 2i, 2i+1} (768 tokens). No collectives; host concatenates the slices.

Per-core device pipeline (all matmuls bf16 with fp32 PSUM accumulation):
  1. QKV projection in transposed layout: QT/KT = W @ x^T (channels on
     partitions), V in natural [token, dh] layout with a ones-column per head
     appended (so the attention-value matmul also produces softmax sums).
  2. Scores computed transposed, S^T[k, q] = K @ Q^T, per head per key-tile.
  3. exp on ScalarE straight out of PSUM (softmax scale folded into the
     activation's `scale`; no max-subtraction needed: scores are O(10) so
     fp32 exp cannot overflow; this matches jax softmax to rounding error).
  4. Mask structure applied multiplicatively to the bf16 probabilities:
     whole-block validity flags (per-core data) and a tril mask for the
     diagonal frame.
  5. O^T = V'^T @ P^T accumulated over key tiles; row 64 of the accumulator
     holds the softmax denominators; normalize via DVE reciprocal + GpSimd
     partition-broadcast.
  6. out^T = Wproj^T.T @ O^T (+bias), DMA out. Host transposes + concats.
"""

from contextlib import ExitStack

import numpy as np
import ml_dtypes

N, C, H, DH = 4096, 512, 8, 64
F = 256                 # frame_seqlen
NCORES = 8
NQ = N // NCORES        # 512 queries per core (2 frames)
KS = 512                # scale tokens (frames 0,1)
KW = 3 * F              # window tokens per core
NK = KS + KW            # 1280 keys per core
BF16 = ml_dtypes.bfloat16

_CACHE = {}


def _build(repeat=1):
    """Build + compile the (single, SPMD) Bass program. Returns nc."""
    import concourse.bass as bass  # noqa: F401
    import concourse.mybir as mybir
    import concourse.tile as tile
    from concourse import bacc

    f32 = mybir.dt.float32
    bf16 = mybir.dt.bfloat16
    EXP = mybir.ActivationFunctionType.Exp

    nc = bacc.Bacc("TRN2", target_bir_lowering=False, debug=False)

    xT = nc.dram_tensor("xT", [C, NK], bf16, kind="ExternalInput")
    wqT = nc.dram_tensor("wqT", [C, C], bf16, kind="ExternalInput")
    wkT = nc.dram_tensor("wkT", [C, C], bf16, kind="ExternalInput")
    wvT = nc.dram_tensor("wvT", [C, C], bf16, kind="ExternalInput")
    wpT = nc.dram_tensor("wpT", [64, H * C], bf16, kind="ExternalInput")
    btab = nc.dram_tensor("btab", [128, 12], f32, kind="ExternalInput")
    dmsk = nc.dram_tensor("dmsk", [8 * 128, F], bf16, kind="ExternalInput")
    outT = nc.dram_tensor("outT", [C, NQ], f32, kind="ExternalOutput")

    with tile.TileContext(nc) as tc, ExitStack() as ctx:
        cp = ctx.enter_context(tc.tile_pool(name="const", bufs=1))
        dp = ctx.enter_context(tc.tile_pool(name="data", bufs=1))
        ptp = ctx.enter_context(tc.tile_pool(name="pt", bufs=8))
        recp = ctx.enter_context(tc.tile_pool(name="rec", bufs=2))
        ppp = ctx.enter_context(tc.tile_pool(name="pp", bufs=2, space="PSUM"))
        stp = ctx.enter_context(tc.tile_pool(name="st", bufs=6, space="PSUM"))

        def body():
            xs = cp.tile([128, 4, NK], bf16, tag="xs")
            wq = cp.tile([128, 4, C], bf16, tag="wq")
            wk = cp.tile([128, 4, C], bf16, tag="wk")
            wv = cp.tile([128, 4, C], bf16, tag="wv")
            xr = xT.ap().rearrange("(a p) t -> p a t", p=128)
            wqr = wqT.ap().rearrange("(a p) o -> p a o", p=128)
            wkr = wkT.ap().rearrange("(a p) o -> p a o", p=128)
            wvr = wvT.ap().rearrange("(a p) o -> p a o", p=128)
            for ci in range(4):
                nc.sync.dma_start(xs[:, ci, :], xr[:, ci, :])
                nc.sync.dma_start(wv[:, ci, :], wvr[:, ci, :])
                nc.sync.dma_start(wq[:, ci, :], wqr[:, ci, :])
                nc.sync.dma_start(wk[:, ci, :], wkr[:, ci, :])
            wp = cp.tile([64, H, C], bf16, tag="wp")
            nc.sync.dma_start(wp[:], wpT.ap().rearrange("p (h o) -> p h o", h=H))
            bt = cp.tile([128, 12], f32, tag="bt")
            nc.sync.dma_start(bt[:], btab.ap())
            dm = cp.tile([128, 8, F], bf16, tag="dm")
            nc.sync.dma_start(dm[:], dmsk.ap().rearrange("(a p) q -> p a q", p=128))

            QT = dp.tile([128, 4, NQ], bf16, tag="QT")
            KT = dp.tile([128, 4, NK], bf16, tag="KT")
            V = dp.tile([128, 10, H, DH + 1], bf16, tag="V")
            OT = dp.tile([128, H, NQ], bf16, tag="OT")
            oT = dp.tile([128, 4, NQ], f32, tag="oT")

            # ---- V projection (natural layout) + ones column ----
            def qkv_psum(n):
                # st pool is idle during the projection phase; steal its slots
                return (ppp.tile([128, 512], f32, tag="pp", name="qps")
                        if n % 2 == 0
                        else stp.tile([128, 512], f32, tag="st", name="qps"))

            nc.vector.memset(V[:, :, :, DH:DH + 1], 1.0)
            for tt in range(10):
                ps = qkv_psum(tt)
                for ci in range(4):
                    nc.tensor.matmul(ps[:], lhsT=xs[:, ci, 128 * tt:128 * (tt + 1)],
                                     rhs=wv[:, ci, :], start=(ci == 0), stop=(ci == 3))
                nc.scalar.copy(V[:, tt, :, 0:DH],
                               ps[:].rearrange("p (h d) -> p h d", h=H))

            # ---- per-pair Q^T/K^T projection interleaved with attention:
            # o-tile `p` of QT/KT is exactly head pair (2p, 2p+1), so emit the
            # pair's projections just before its heads; ACT exps overlap the
            # next pair's projection matmuls on the PE.
            for p in range(4):
                ps = qkv_psum(p)
                for ci in range(4):
                    nc.tensor.matmul(ps[:], lhsT=wq[:, ci, 128 * p:128 * (p + 1)],
                                     rhs=xs[:, ci, KS + F:KS + F + NQ],
                                     start=(ci == 0), stop=(ci == 3))
                nc.vector.tensor_scalar_add(QT[:, p, :], ps[:], bt[:, p:p + 1])
                for ti, (t0, t1) in enumerate(((0, 512), (512, 1024), (1024, 1280))):
                    ps = qkv_psum(p + ti)
                    for ci in range(4):
                        nc.tensor.matmul(ps[:, 0:t1 - t0],
                                         lhsT=wk[:, ci, 128 * p:128 * (p + 1)],
                                         rhs=xs[:, ci, t0:t1],
                                         start=(ci == 0), stop=(ci == 3))
                    nc.vector.tensor_scalar_add(KT[:, p, t0:t1], ps[:, 0:t1 - t0],
                                                bt[:, 4 + p:5 + p])
                for h in (2 * p, 2 * p + 1):
                    po = h // 2
                    prow = slice((h % 2) * 64, (h % 2) * 64 + 64)
                    qh = QT[prow, po, :]
                    # scale keys: k-tiles 0..3, all queries
                    pts = []
                    for kt in range(4):
                        st = stp.tile([128, 512], f32, tag="st", name="st")
                        nc.tensor.matmul(st[:],
                                         lhsT=KT[prow, po, 128 * kt:128 * (kt + 1)],
                                         rhs=qh, start=True, stop=True)
                        pt = ptp.tile([128, NQ], bf16, tag="pts", name="pt")
                        nc.scalar.activation(pt[:], st[:], EXP,
                                             scale=float(DH) ** -0.5)
                        pts.append(pt)
                    # window keys: query-frame r, k-tile pairs, masked in halves
                    ptws = []
                    for r in range(2):
                        for g in range(2):
                            stw = stp.tile([128, 2, 256], f32, tag="st", name="stw")
                            for j in range(2):
                                kt = 4 + 2 * r + 2 * g + j
                                nc.tensor.matmul(
                                    stw[:, j, :],
                                    lhsT=KT[prow, po, 128 * kt:128 * (kt + 1)],
                                    rhs=qh[:, 256 * r:256 * (r + 1)],
                                    start=True, stop=True)
                            ptw = ptp.tile([128, 2, 256], bf16, tag="ptw", name="ptw")
                            nc.scalar.activation(ptw[:], stw[:], EXP,
                                                 scale=float(DH) ** -0.5)
                            nc.vector.tensor_mul(
                                ptw[:], ptw[:],
                                dm[:, 4 * r + 2 * g:4 * r + 2 * g + 2, :])
                            ptws.append(ptw)
                    # O^T accumulation (+ sums in row 64 via the ones column)
                    av = ppp.tile([128, 512], f32, tag="pp", name="av")
                    for kt in range(4):
                        nc.tensor.matmul(av[0:65, :], lhsT=V[:, kt, h, :],
                                         rhs=pts[kt][:],
                                         start=(kt == 0), stop=False,
                                         skip_group_check=True)
                    for r in range(2):
                        for g in range(2):
                            for j in range(2):
                                kt = 4 + 2 * r + 2 * g + j
                                nc.tensor.matmul(av[0:65, 256 * r:256 * (r + 1)],
                                                 lhsT=V[:, kt, h, :],
                                                 rhs=ptws[2 * r + g][:, j, :],
                                                 start=False,
                                                 stop=(g == 1 and j == 1),
                                                 skip_group_check=True)
                    # stage unnormalized O^T; normalize via a [128, 4]-reshaped
                    # reciprocal (a [1, 512] reciprocal runs on one lane,
                    # ~3.4us)
                    nc.vector.tensor_copy(OT[0:64, h, :], av[0:64, :])
                    sm = recp.tile([65, NQ], f32, tag="sm", name="sm")
                    nc.vector.tensor_copy(sm[64:65, :], av[64:65, :])
                    rs = recp.tile([128, 4], f32, tag="rs", name="rs")
                    nc.sync.dma_start(rs[:], sm[64:65, :])
                    nc.vector.reciprocal(rs[:], rs[:])
                    rcb = recp.tile([64, NQ], f32, tag="rcb", name="rcb")
                    nc.sync.dma_start(rcb[0:1, :], rs[:])
                    nc.gpsimd.partition_broadcast(rcb[:, :], rcb[0:1, :])
                    nc.vector.tensor_mul(OT[0:64, h, :], OT[0:64, h, :],
                                         rcb[0:64, :])

            # ---- output projection ----
            od = outT.ap().rearrange("(a p) q -> p a q", p=128)
            for ot in range(4):
                pj = ppp.tile([128, 512], f32, tag="pp")
                for h in range(8):
                    nc.tensor.matmul(pj[:],
                                     lhsT=wp[0:64, h, 128 * ot:128 * (ot + 1)],
                                     rhs=OT[0:64, h, :], start=(h == 0), stop=(h == 7))
                nc.vector.tensor_scalar_add(oT[:, ot, :], pj[:], bt[:, 8 + ot:9 + ot])
                nc.sync.dma_start(od[:, ot, :], oT[:, ot, :])

        if repeat == 1:
            body()
        else:
            with tc.For_i(0, repeat, 1):
                body()

    nc.compile()
    return nc


def _get_nc(repeat=1):
    key = ("nc", repeat)
    if key not in _CACHE:
        _CACHE[key] = _build(repeat)
    return _CACHE[key]


def _host_prep(x, qkv_w, qkv_b, proj_w, proj_b):
    """Build the 8 per-core input maps."""
    x = np.asarray(x, np.float32).reshape(N, C)
    qkv_w = np.asarray(qkv_w, np.float32)
    qkv_b = np.asarray(qkv_b, np.float32)
    proj_w = np.asarray(proj_w, np.float32)
    proj_b = np.asarray(proj_b, np.float32)

    xs_bf = x.astype(BF16)
    xT_scale = np.ascontiguousarray(xs_bf[0:KS].T)            # [C, 512]
    wqT = np.ascontiguousarray(qkv_w[0:C].T.astype(BF16))
    wkT = np.ascontiguousarray(qkv_w[C:2 * C].T.astype(BF16))
    wvT = np.ascontiguousarray(qkv_w[2 * C:3 * C].T.astype(BF16))
    # head-chunked proj weight: wpT[p, h*512+o] = proj_w[o, h*64+p] so every
    # head's 64-row contraction chunk sits at partition base 0
    wpT = np.ascontiguousarray(
        proj_w.T.reshape(H, 64, C).transpose(1, 0, 2).reshape(64, H * C).astype(BF16))

    # value-bias folds through normalized attention into the proj bias:
    # O = sum_k phat_k (V_k + vb) = O_hat + vb, so out += vb @ proj_w.T
    pb_eff = proj_b + qkv_b[2 * C:3 * C] @ proj_w.T
    btab = np.zeros((128, 12), np.float32)
    for ot in range(4):
        btab[:, ot] = qkv_b[0:C][128 * ot:128 * (ot + 1)]
        btab[:, 4 + ot] = qkv_b[C:2 * C][128 * ot:128 * (ot + 1)]
        btab[:, 8 + ot] = pb_eff[128 * ot:128 * (ot + 1)]

    # tril01[j, q] = 1 if key j <= query q (within the same frame)
    tril01 = (np.arange(F)[:, None] <= np.arange(F)[None, :])

    in_maps = []
    for i in range(NCORES):
        win = np.zeros((KW, C), BF16)
        lo = F * (2 * i - 1)
        src = xs_bf[max(0, lo):F * (2 * i + 2)]
        win[KW - len(src):] = src
        xTi = np.empty((C, NK), BF16)
        xTi[:, 0:KS] = xT_scale
        xTi[:, KS:] = win.T

        vf = np.array([1.0 if (2 * i - 1) >= 2 else 0.0,
                       1.0 if (2 * i) >= 2 else 0.0], np.float32)
        vd = np.array([1.0 if (2 * i) >= 2 else 0.0,
                       1.0 if (2 * i + 1) >= 2 else 0.0], np.float32)
        dmsk = np.zeros((8 * 128, F), np.float32)
        for r in range(2):
            for j in range(4):
                blk = dmsk[(4 * r + j) * 128:(4 * r + j + 1) * 128]
                if j < 2:
                    blk[:] = vf[r]
                else:
                    kh = j - 2
                    blk[:] = tril01[128 * kh:128 * (kh + 1), :] * vd[r]
        in_maps.append({
            "xT": xTi, "wqT": wqT, "wkT": wkT, "wvT": wvT, "wpT": wpT,
            "btab": btab,
            "dmsk": dmsk.astype(BF16),
        })
    return in_maps


def _check_fixed_params(block_mask, video_mask, frame_seqlen,
                        sliding_window_size, num_frame_per_block,
                        num_frame_for_scale):
    if int(frame_seqlen) != F or int(sliding_window_size) != 2 \
            or int(num_frame_per_block) != 1 or int(num_frame_for_scale) != 2:
        return False
    vm = np.asarray(video_mask)
    if not bool(vm.all()):
        return False
    bm = np.asarray(block_mask)
    if bm.shape != (N, N):
        return False
    # spot-check causality structure of block_mask (full check is 16M bools)
    idx = np.linspace(0, N - 1, 64).astype(int)
    sub = bm[np.ix_(idx, idx)]
    if not np.array_equal(sub, np.tril(np.ones_like(sub))):
        return False
    return True


def _numpy_reference(x, block_mask, video_mask, qkv_w, qkv_b, proj_w, proj_b,
                     frame_seqlen, sliding_window_size, num_frame_per_block,
                     num_frame_for_scale):
    """Fallback: direct numpy evaluation of the reference semantics."""
    x = np.asarray(x, np.float32)
    b, n, c = x.shape
    dh = c // H
    qkv = (x @ np.asarray(qkv_w).T + np.asarray(qkv_b)).reshape(b, n, 3, H, dh)
    qkv = qkv.transpose(2, 0, 3, 1, 4)
    q, k, v = qkv[0], qkv[1], qkv[2]
    mask = np.asarray(block_mask)[:n, :n][None, None]
    vm = np.asarray(video_mask)[:, None, None, None]
    mask = mask | ~vm
    fs = int(frame_seqlen)
    if int(sliding_window_size) > 0 and fs is not None:
        f = np.arange(n) // fs
        w = int(sliding_window_size) * int(num_frame_per_block)
        sliding = (f[None, :] <= f[:, None]) & (f[None, :] >= f[:, None] - w + 1)
        mask = mask & sliding[None, None]
        if int(num_frame_for_scale) > 0:
            s = int(num_frame_for_scale) * fs
            mask = mask.copy()
            mask[:, :, :, :s] = True
    scores = np.einsum('bhqd,bhkd->bhqk', q, k) * (dh ** -0.5)
    scores = np.where(mask, scores, np.float32(-1e30))
    scores -= scores.max(axis=-1, keepdims=True)
    e = np.exp(scores)
    attn = e / e.sum(axis=-1, keepdims=True)
    o = np.einsum('bhqk,bhkd->bhqd', attn, v)
    o = o.transpose(0, 2, 1, 3).reshape(b, n, c)
    return (o @ np.asarray(proj_w).T + np.asarray(proj_b)).astype(np.float32)


def kernel(x, block_mask, video_mask, qkv_w, qkv_b, proj_w, proj_b,
           frame_seqlen, sliding_window_size, num_frame_per_block,
           num_frame_for_scale):
    if not _check_fixed_params(block_mask, video_mask, frame_seqlen,
                               sliding_window_size, num_frame_per_block,
                               num_frame_for_scale):
        return _numpy_reference(x, block_mask, video_mask, qkv_w, qkv_b,
                                proj_w, proj_b, frame_seqlen,
                                sliding_window_size, num_frame_per_block,
                                num_frame_for_scale)

    from concourse.bass_utils import run_bass_kernel_spmd

    nc = _get_nc()
    in_maps = _host_prep(x, qkv_w, qkv_b, proj_w, proj_b)
    res = run_bass_kernel_spmd(nc, in_maps, core_ids=list(range(NCORES)))
    out = np.empty((N, C), np.float32)
    for i in range(NCORES):
        out[NQ * i:NQ * (i + 1)] = res.results[i]["outT"].T
    return out.reshape(1, N, C)


# revision 15
# speedup vs baseline: 1.2152x; 1.0182x over previous
"""Trainium2 Bass kernel for CausalAttention (sliding-window + scale-frame sparse attention).

Problem shape (hardcoded): B=1, N=4096, C=512, H=8, Dh=64, frame_seqlen=256,
sliding_window_size=2, num_frame_per_block=1, num_frame_for_scale=2.

Sharding: sequence-parallel over 8 NeuronCores. Core i owns queries
[512*i, 512*(i+1)) (= frames 2i, 2i+1) and returns that slice of the final
output. Keys needed per core: the 512 "scale" tokens (frames 0,1; attended by
every query unconditionally per the reference mask) plus a 3-frame window
{2i-1, 2i, 2i+1} (768 tokens). No collectives; host concatenates the slices.

Per-core device pipeline (all matmuls bf16 with fp32 PSUM accumulation):
  1. QKV projection in transposed layout: QT/KT = W @ x^T (channels on
     partitions), V in natural [token, dh] layout with a ones-column per head
     appended (so the attention-value matmul also produces softmax sums).
  2. Scores computed transposed, S^T[k, q] = K @ Q^T, per head per key-tile.
  3. exp on ScalarE straight out of PSUM (softmax scale folded into the
     activation's `scale`; no max-subtraction needed: scores are O(10) so
     fp32 exp cannot overflow; this matches jax softmax to rounding error).
  4. Mask structure applied multiplicatively to the bf16 probabilities:
     whole-block validity flags (per-core data) and a tril mask for the
     diagonal frame.
  5. O^T = V'^T @ P^T accumulated over key tiles; row 64 of the accumulator
     holds the softmax denominators; normalize via DVE reciprocal + GpSimd
     partition-broadcast.
  6. out^T = Wproj^T.T @ O^T (+bias), DMA out. Host transposes + concats.
"""

from contextlib import ExitStack

import numpy as np
import ml_dtypes

N, C, H, DH = 4096, 512, 8, 64
F = 256                 # frame_seqlen
NCORES = 8
NQ = N // NCORES        # 512 queries per core (2 frames)
KS = 512                # scale tokens (frames 0,1)
KW = 3 * F              # window tokens per core
NK = KS + KW            # 1280 keys per core
BF16 = ml_dtypes.bfloat16

_CACHE = {}


def _build(repeat=1):
    """Build + compile the (single, SPMD) Bass program. Returns nc."""
    import concourse.bass as bass  # noqa: F401
    import concourse.mybir as mybir
    import concourse.tile as tile
    from concourse import bacc

    f32 = mybir.dt.float32
    bf16 = mybir.dt.bfloat16
    EXP = mybir.ActivationFunctionType.Exp

    nc = bacc.Bacc("TRN2", target_bir_lowering=False, debug=False)

    xT = nc.dram_tensor("xT", [C, NK], bf16, kind="ExternalInput")
    wqT = nc.dram_tensor("wqT", [C, C], bf16, kind="ExternalInput")
    wkT = nc.dram_tensor("wkT", [C, C], bf16, kind="ExternalInput")
    wvT = nc.dram_tensor("wvT", [C, C], bf16, kind="ExternalInput")
    wpT = nc.dram_tensor("wpT", [64, H * C], bf16, kind="ExternalInput")
    btab = nc.dram_tensor("btab", [128, 12], f32, kind="ExternalInput")
    dmsk = nc.dram_tensor("dmsk", [128, 2048], bf16, kind="ExternalInput")
    outT = nc.dram_tensor("outT", [C, NQ], f32, kind="ExternalOutput")

    with tile.TileContext(nc) as tc, ExitStack() as ctx:
        cp = ctx.enter_context(tc.tile_pool(name="const", bufs=1))
        dp = ctx.enter_context(tc.tile_pool(name="data", bufs=1))
        ptp = ctx.enter_context(tc.tile_pool(name="pt", bufs=8))
        recp = ctx.enter_context(tc.tile_pool(name="rec", bufs=2))
        ppp = ctx.enter_context(tc.tile_pool(name="pp", bufs=2, space="PSUM"))
        stp = ctx.enter_context(tc.tile_pool(name="st", bufs=6, space="PSUM"))

        def body():
            xs = cp.tile([128, 4, NK], bf16, tag="xs")
            wq = cp.tile([128, 4, C], bf16, tag="wq")
            wk = cp.tile([128, 4, C], bf16, tag="wk")
            wv = cp.tile([128, 4, C], bf16, tag="wv")
            xr = xT.ap().rearrange("(a p) t -> p a t", p=128)
            wqr = wqT.ap().rearrange("(a p) o -> p a o", p=128)
            wkr = wkT.ap().rearrange("(a p) o -> p a o", p=128)
            wvr = wvT.ap().rearrange("(a p) o -> p a o", p=128)
            for ci in range(4):
                nc.sync.dma_start(xs[:, ci, :], xr[:, ci, :])
                nc.sync.dma_start(wv[:, ci, :], wvr[:, ci, :])
                nc.sync.dma_start(wq[:, ci, :], wqr[:, ci, :])
                nc.sync.dma_start(wk[:, ci, :], wkr[:, ci, :])
            wp = cp.tile([64, H, C], bf16, tag="wp")
            nc.sync.dma_start(wp[:], wpT.ap().rearrange("p (h o) -> p h o", h=H))
            bt = cp.tile([128, 12], f32, tag="bt")
            nc.sync.dma_start(bt[:], btab.ap())
            dm = cp.tile([128, 2048], bf16, tag="dm")
            nc.sync.dma_start(dm[:], dmsk.ap())

            QT = dp.tile([128, 4, NQ], bf16, tag="QT")
            KT = dp.tile([128, 4, NK], bf16, tag="KT")
            V = dp.tile([128, 10, H, DH + 1], bf16, tag="V")
            OT = dp.tile([128, H, NQ], bf16, tag="OT")
            oT = dp.tile([128, 4, NQ], f32, tag="oT")

            # ---- V projection (natural layout) + ones column ----
            def qkv_psum(n):
                # st pool is idle during the projection phase; steal its slots
                return (ppp.tile([128, 512], f32, tag="pp", name="qps")
                        if n % 2 == 0
                        else stp.tile([128, 512], f32, tag="st", name="qps"))

            nc.vector.memset(V[:, :, :, DH:DH + 1], 1.0)
            for tt in range(10):
                ps = qkv_psum(tt)
                for ci in range(4):
                    nc.tensor.matmul(ps[:], lhsT=xs[:, ci, 128 * tt:128 * (tt + 1)],
                                     rhs=wv[:, ci, :], start=(ci == 0), stop=(ci == 3))
                nc.scalar.copy(V[:, tt, :, 0:DH],
                               ps[:].rearrange("p (h d) -> p h d", h=H))

            # ---- per-pair Q^T/K^T projection interleaved with attention:
            # o-tile `p` of QT/KT is exactly head pair (2p, 2p+1), so emit the
            # pair's projections just before its heads; ACT exps overlap the
            # next pair's projection matmuls on the PE.
            for p in range(4):
                ps = qkv_psum(p)
                for ci in range(4):
                    nc.tensor.matmul(ps[:], lhsT=wq[:, ci, 128 * p:128 * (p + 1)],
                                     rhs=xs[:, ci, KS + F:KS + F + NQ],
                                     start=(ci == 0), stop=(ci == 3))
                nc.vector.tensor_scalar_add(QT[:, p, :], ps[:], bt[:, p:p + 1])
                for ti, (t0, t1) in enumerate(((0, 512), (512, 1024), (1024, 1280))):
                    ps = qkv_psum(p + ti)
                    for ci in range(4):
                        nc.tensor.matmul(ps[:, 0:t1 - t0],
                                         lhsT=wk[:, ci, 128 * p:128 * (p + 1)],
                                         rhs=xs[:, ci, t0:t1],
                                         start=(ci == 0), stop=(ci == 3))
                    nc.vector.tensor_scalar_add(KT[:, p, t0:t1], ps[:, 0:t1 - t0],
                                                bt[:, 4 + p:5 + p])
                for h in (2 * p, 2 * p + 1):
                    po = h // 2
                    prow = slice((h % 2) * 64, (h % 2) * 64 + 64)
                    qh = QT[prow, po, :]
                    # scale keys (k-tiles 0..3): full query range, no mask
                    pts = []
                    for kt in range(4):
                        st = stp.tile([128, 512], f32, tag="st", name="st")
                        nc.tensor.matmul(st[:],
                                         lhsT=KT[prow, po, 128 * kt:128 * (kt + 1)],
                                         rhs=qh, start=True, stop=True)
                        pt = ptp.tile([128, NQ], bf16, tag="pts", name="pt")
                        nc.scalar.activation(pt[:], st[:], EXP,
                                             scale=float(DH) ** -0.5)
                        pts.append(pt)
                    # window keys. k-tiles 6,7 (the queries' own even frame) are
                    # computed once over the full query range — the mask holds
                    # tril*vd for the first query frame and the full-block flag
                    # for the second — so no k-tile is scored twice.
                    def masked_scores(kts, q0, q1, mcol):
                        n = len(kts)
                        stw = stp.tile([128, n, q1 - q0], f32, tag="st", name="stw")
                        for j, kt in enumerate(kts):
                            nc.tensor.matmul(stw[:, j, :],
                                             lhsT=KT[prow, po, 128 * kt:128 * (kt + 1)],
                                             rhs=qh[:, q0:q1], start=True, stop=True)
                        pw = ptp.tile([128, n, q1 - q0], bf16, tag="ptw", name="pw")
                        nc.scalar.activation(pw[:], stw[:], EXP,
                                             scale=float(DH) ** -0.5)
                        w = n * (q1 - q0)
                        nc.vector.tensor_mul(
                            pw[:], pw[:],
                            dm[:, mcol:mcol + w].rearrange(
                                "p (a q) -> p a q", a=n))
                        return pw
                    pw45 = masked_scores([4, 5], 0, 256, 0)
                    pw6 = masked_scores([6], 0, 512, 512)
                    pw7 = masked_scores([7], 0, 512, 1024)
                    pw89 = masked_scores([8, 9], 256, 512, 1536)
                    # O^T accumulation (+ sums in row 64 via the ones column)
                    av = ppp.tile([128, 512], f32, tag="pp", name="av")
                    for kt in range(4):
                        nc.tensor.matmul(av[0:65, :], lhsT=V[:, kt, h, :],
                                         rhs=pts[kt][:],
                                         start=(kt == 0), stop=False,
                                         skip_group_check=True)
                    for j, kt in enumerate((6, 7)):
                        nc.tensor.matmul(av[0:65, :], lhsT=V[:, kt, h, :],
                                         rhs=(pw6 if j == 0 else pw7)[:, 0, :],
                                         start=False, stop=False,
                                         skip_group_check=True)
                    for j, kt in enumerate((4, 5)):
                        nc.tensor.matmul(av[0:65, 0:256], lhsT=V[:, kt, h, :],
                                         rhs=pw45[:, j, :],
                                         start=False, stop=(j == 1),
                                         skip_group_check=True)
                    for j, kt in enumerate((8, 9)):
                        nc.tensor.matmul(av[0:65, 256:512], lhsT=V[:, kt, h, :],
                                         rhs=pw89[:, j, :],
                                         start=False, stop=(j == 1),
                                         skip_group_check=True)
                    # stage unnormalized O^T; normalize via a [128, 4]-reshaped
                    # reciprocal (a [1, 512] reciprocal runs on one lane,
                    # ~3.4us)
                    nc.vector.tensor_copy(OT[0:64, h, :], av[0:64, :])
                    sm = recp.tile([65, NQ], f32, tag="sm", name="sm")
                    nc.vector.tensor_copy(sm[64:65, :], av[64:65, :])
                    rs = recp.tile([128, 4], f32, tag="rs", name="rs")
                    nc.sync.dma_start(rs[:], sm[64:65, :])
                    nc.vector.reciprocal(rs[:], rs[:])
                    rcb = recp.tile([64, NQ], f32, tag="rcb", name="rcb")
                    nc.sync.dma_start(rcb[0:1, :], rs[:])
                    nc.gpsimd.partition_broadcast(rcb[:, :], rcb[0:1, :])
                    nc.vector.tensor_mul(OT[0:64, h, :], OT[0:64, h, :],
                                         rcb[0:64, :])

            # ---- output projection ----
            od = outT.ap().rearrange("(a p) q -> p a q", p=128)
            for ot in range(4):
                pj = ppp.tile([128, 512], f32, tag="pp")
                for h in range(8):
                    nc.tensor.matmul(pj[:],
                                     lhsT=wp[0:64, h, 128 * ot:128 * (ot + 1)],
                                     rhs=OT[0:64, h, :], start=(h == 0), stop=(h == 7))
                nc.vector.tensor_scalar_add(oT[:, ot, :], pj[:], bt[:, 8 + ot:9 + ot])
                nc.sync.dma_start(od[:, ot, :], oT[:, ot, :])

        if repeat == 1:
            body()
        else:
            with tc.For_i(0, repeat, 1):
                body()

    nc.compile()
    return nc


def _get_nc(repeat=1):
    key = ("nc", repeat)
    if key not in _CACHE:
        _CACHE[key] = _build(repeat)
    return _CACHE[key]


def _host_prep(x, qkv_w, qkv_b, proj_w, proj_b):
    """Build the 8 per-core input maps."""
    x = np.asarray(x, np.float32).reshape(N, C)
    qkv_w = np.asarray(qkv_w, np.float32)
    qkv_b = np.asarray(qkv_b, np.float32)
    proj_w = np.asarray(proj_w, np.float32)
    proj_b = np.asarray(proj_b, np.float32)

    xs_bf = x.astype(BF16)
    xT_scale = np.ascontiguousarray(xs_bf[0:KS].T)            # [C, 512]
    wqT = np.ascontiguousarray(qkv_w[0:C].T.astype(BF16))
    wkT = np.ascontiguousarray(qkv_w[C:2 * C].T.astype(BF16))
    wvT = np.ascontiguousarray(qkv_w[2 * C:3 * C].T.astype(BF16))
    # head-chunked proj weight: wpT[p, h*512+o] = proj_w[o, h*64+p] so every
    # head's 64-row contraction chunk sits at partition base 0
    wpT = np.ascontiguousarray(
        proj_w.T.reshape(H, 64, C).transpose(1, 0, 2).reshape(64, H * C).astype(BF16))

    # value-bias folds through normalized attention into the proj bias:
    # O = sum_k phat_k (V_k + vb) = O_hat + vb, so out += vb @ proj_w.T
    pb_eff = proj_b + qkv_b[2 * C:3 * C] @ proj_w.T
    btab = np.zeros((128, 12), np.float32)
    for ot in range(4):
        btab[:, ot] = qkv_b[0:C][128 * ot:128 * (ot + 1)]
        btab[:, 4 + ot] = qkv_b[C:2 * C][128 * ot:128 * (ot + 1)]
        btab[:, 8 + ot] = pb_eff[128 * ot:128 * (ot + 1)]

    # tril01[j, q] = 1 if key j <= query q (within the same frame)
    tril01 = (np.arange(F)[:, None] <= np.arange(F)[None, :])

    in_maps = []
    for i in range(NCORES):
        win = np.zeros((KW, C), BF16)
        lo = F * (2 * i - 1)
        src = xs_bf[max(0, lo):F * (2 * i + 2)]
        win[KW - len(src):] = src
        xTi = np.empty((C, NK), BF16)
        xTi[:, 0:KS] = xT_scale
        xTi[:, KS:] = win.T

        vf = np.array([1.0 if (2 * i - 1) >= 2 else 0.0,
                       1.0 if (2 * i) >= 2 else 0.0], np.float32)
        vd = np.array([1.0 if (2 * i) >= 2 else 0.0,
                       1.0 if (2 * i + 1) >= 2 else 0.0], np.float32)
        dmsk = np.zeros((128, 2048), np.float32)
        # kt4, kt5 (frame 2i-1, first-query-frame keys): full-block flag
        dmsk[:, 0:512] = vf[0]
        # kt6, kt7 (frame 2i): tril*vd[0] for query frame r=0, flag vf[1]
        # (== vd[0]) for r=1
        for kh in range(2):
            blk = dmsk[:, 512 + 512 * kh:1024 + 512 * kh]
            blk[:, 0:256] = tril01[128 * kh:128 * (kh + 1), :] * vd[0]
            blk[:, 256:512] = vd[0]
        # kt8, kt9 (frame 2i+1): tril*vd[1] for query frame r=1
        for kh in range(2):
            dmsk[:, 1536 + 256 * kh:1792 + 256 * kh] = (
                tril01[128 * kh:128 * (kh + 1), :] * vd[1])
        in_maps.append({
            "xT": xTi, "wqT": wqT, "wkT": wkT, "wvT": wvT, "wpT": wpT,
            "btab": btab,
            "dmsk": dmsk.astype(BF16),
        })
    return in_maps


def _check_fixed_params(block_mask, video_mask, frame_seqlen,
                        sliding_window_size, num_frame_per_block,
                        num_frame_for_scale):
    if int(frame_seqlen) != F or int(sliding_window_size) != 2 \
            or int(num_frame_per_block) != 1 or int(num_frame_for_scale) != 2:
        return False
    vm = np.asarray(video_mask)
    if not bool(vm.all()):
        return False
    bm = np.asarray(block_mask)
    if bm.shape != (N, N):
        return False
    # spot-check causality structure of block_mask (full check is 16M bools)
    idx = np.linspace(0, N - 1, 64).astype(int)
    sub = bm[np.ix_(idx, idx)]
    if not np.array_equal(sub, np.tril(np.ones_like(sub))):
        return False
    return True


def _numpy_reference(x, block_mask, video_mask, qkv_w, qkv_b, proj_w, proj_b,
                     frame_seqlen, sliding_window_size, num_frame_per_block,
                     num_frame_for_scale):
    """Fallback: direct numpy evaluation of the reference semantics."""
    x = np.asarray(x, np.float32)
    b, n, c = x.shape
    dh = c // H
    qkv = (x @ np.asarray(qkv_w).T + np.asarray(qkv_b)).reshape(b, n, 3, H, dh)
    qkv = qkv.transpose(2, 0, 3, 1, 4)
    q, k, v = qkv[0], qkv[1], qkv[2]
    mask = np.asarray(block_mask)[:n, :n][None, None]
    vm = np.asarray(video_mask)[:, None, None, None]
    mask = mask | ~vm
    fs = int(frame_seqlen)
    if int(sliding_window_size) > 0 and fs is not None:
        f = np.arange(n) // fs
        w = int(sliding_window_size) * int(num_frame_per_block)
        sliding = (f[None, :] <= f[:, None]) & (f[None, :] >= f[:, None] - w + 1)
        mask = mask & sliding[None, None]
        if int(num_frame_for_scale) > 0:
            s = int(num_frame_for_scale) * fs
            mask = mask.copy()
            mask[:, :, :, :s] = True
    scores = np.einsum('bhqd,bhkd->bhqk', q, k) * (dh ** -0.5)
    scores = np.where(mask, scores, np.float32(-1e30))
    scores -= scores.max(axis=-1, keepdims=True)
    e = np.exp(scores)
    attn = e / e.sum(axis=-1, keepdims=True)
    o = np.einsum('bhqk,bhkd->bhqd', attn, v)
    o = o.transpose(0, 2, 1, 3).reshape(b, n, c)
    return (o @ np.asarray(proj_w).T + np.asarray(proj_b)).astype(np.float32)


def kernel(x, block_mask, video_mask, qkv_w, qkv_b, proj_w, proj_b,
           frame_seqlen, sliding_window_size, num_frame_per_block,
           num_frame_for_scale):
    if not _check_fixed_params(block_mask, video_mask, frame_seqlen,
                               sliding_window_size, num_frame_per_block,
                               num_frame_for_scale):
        return _numpy_reference(x, block_mask, video_mask, qkv_w, qkv_b,
                                proj_w, proj_b, frame_seqlen,
                                sliding_window_size, num_frame_per_block,
                                num_frame_for_scale)

    from concourse.bass_utils import run_bass_kernel_spmd

    nc = _get_nc()
    in_maps = _host_prep(x, qkv_w, qkv_b, proj_w, proj_b)
    res = run_bass_kernel_spmd(nc, in_maps, core_ids=list(range(NCORES)))
    out = np.empty((N, C), np.float32)
    for i in range(NCORES):
        out[NQ * i:NQ * (i + 1)] = res.results[i]["outT"].T
    return out.reshape(1, N, C)
